# revision 35
# baseline (speedup 1.0000x reference)
"""Chamfer loss kernel for Trainium2 (8 NeuronCores, data-parallel over batch).

Strategy
--------
loss = (mean_i min_j d2[i,j] + mean_j min_i d2[i,j]) / 2 with
d2[i,j] = |a_i|^2 + |b_j|^2 - 2 a_i.b_j computed per batch (one batch per core).

Device kernel (per core, one batch, N=8192 points, D=3):
  * Host pre-builds augmented K=13 bf16 operands so a single TensorE matmul
    emits exact-enough fp32 d2 tiles straight into PSUM:
      rows 0-8: hi/lo split of -2*a paired with hi/lo of b (bf16 hi/lo
                double-float trick, ~2^-18 relative precision),
      rows 9-12: |a|^2 hi/lo against ones, ones against |b|^2 hi/lo.
  * ScalarE casts each PSUM fp32 chunk to fp16 in SBUF (the 1x-rate copy is
    the critical engine; chunk size 2048 double-buffers the 8 PSUM banks).
  * Row direction: VectorE min-folds the row twice (2x fp16 mode) and the
    [128,2048] partial mins ship to DRAM; host finishes the row min.
  * Column direction: VectorE accumulates a running elementwise-min tile at
    the 2x fp16 perf mode.
  * Host (float64): finish row mins, cross-partition column min, clamp at 0,
    sum, normalize.
"""

import numpy as np
import ml_dtypes

B, N, D = 8, 8192, 3
K = 13          # augmented contraction rows
P = 128         # partitions / i-tile height
JC = 2048       # copy chunk (4 PSUM banks)
MM = 512        # matmul moving free dim (1 PSUM bank)
NT = N // P     # 64 i-tiles
NC_ = N // JC   # 4 chunks per i-tile row
NCHUNKS = NT * NC_  # 256 total copy chunks
N_TTR = 0       # chunks copied by VectorE (TTR) instead of ScalarE
N_DMACOL = 0    # accum-min DMA: rejected by walrus birverifier, keep 0
FOLD = 2        # row-min fold depth on device; r1 ships N >> FOLD wide
BIG = 60000.0   # fp16-safe +inf surrogate (d2 <= ~300 here)

_bf16 = ml_dtypes.bfloat16

_CACHE = {}


def _spread(n, total):
    # spread special-path items evenly
    if n <= 0:
        return set()
    step = total / n
    return {int(step * i + step / 2) for i in range(n)}


def _build_module(repeats=1, n_ttr=N_TTR, n_dmacol=N_DMACOL, jc=JC, fold=FOLD):
    import concourse.bass as bass
    import concourse.tile as tile
    from concourse import bacc, mybir

    nchunk_row = N // jc
    nchunks = NT * nchunk_row
    psum_bufs = 4096 // jc

    nc = bacc.Bacc(
        "TRN2",
        target_bir_lowering=False,
        debug=False,
        enable_asserts=False,
    )

    lhs_d = nc.dram_tensor("lhs", [K, N], mybir.dt.bfloat16, kind="ExternalInput")
    rhs_d = nc.dram_tensor("rhs", [K, N], mybir.dt.bfloat16, kind="ExternalInput")
    cola_d = nc.dram_tensor("colacc_a", [P, N], mybir.dt.float16, kind="ExternalOutput")
    colb_d = (
        nc.dram_tensor("colacc_b", [P, N], mybir.dt.float16, kind="ExternalOutput")
        if n_dmacol > 0
        else None
    )
    r1_d = nc.dram_tensor(
        "r1", [NT, P, N >> fold], mybir.dt.float16, kind="ExternalOutput"
    )

    fp16 = mybir.dt.float16
    f32 = mybir.dt.float32
    mn = mybir.AluOpType.min
    ttr_chunks = _spread(n_ttr, nchunks)
    dmacol_tiles = _spread(n_dmacol, NT)

    with tile.TileContext(nc) as tc:
        with (
            tc.tile_pool(name="const", bufs=1) as const,
            tc.tile_pool(name="srow", bufs=4) as srow_pool,
            tc.tile_pool(name="rtmp", bufs=3) as rtmp_pool,
            tc.tile_pool(name="accs", bufs=2) as accs_pool,
            tc.tile_pool(name="psum", bufs=psum_bufs, space="PSUM") as psum_pool,
        ):
            lhs_sb = const.tile([K, N], mybir.dt.bfloat16)
            rhs_sb = const.tile([K, N], mybir.dt.bfloat16)
            nc.sync.dma_start(lhs_sb[:], lhs_d[:])
            nc.sync.dma_start(rhs_sb[:], rhs_d[:])

            colacc_a = const.tile([P, N], fp16)
            colacc_b = const.tile([P, N], fp16) if n_dmacol > 0 else None
            inf_t = const.tile([P, jc], fp16)
            nc.gpsimd.memset(inf_t[:], BIG)

            # touch ScalarE immediately so its activation-table load and
            # first-op drain hide under the input DMAs
            warm = const.tile([1, 32], fp16)
            nc.vector.memset(warm[:], 0.0)
            nc.scalar.copy(warm[:], warm[:])

            for _rep in range(repeats):
                nc.gpsimd.memset(colacc_a[:], BIG)
                if colacc_b is not None:
                    nc.gpsimd.memset(colacc_b[:], BIG)
                for t in range(NT):
                    lhsT = lhs_sb[:, t * P:(t + 1) * P]
                    s_row = srow_pool.tile([P, N], fp16)
                    for c in range(nchunk_row):
                        ps = psum_pool.tile([P, jc], f32)
                        for q in range(jc // MM):
                            j0 = c * jc + q * MM
                            nc.tensor.matmul(
                                ps[:, q * MM:(q + 1) * MM],
                                lhsT,
                                rhs_sb[:, j0:j0 + MM],
                                start=True,
                                stop=True,
                            )
                        dst = s_row[:, c * jc:(c + 1) * jc]
                        if t * nchunk_row + c in ttr_chunks:
                            # fp32 PSUM -> fp16 SBUF copy on VectorE (engine
                            # balance); the row-min accum is a throwaway.
                            acc_s = accs_pool.tile([P, 1], f32)
                            nc.vector.tensor_tensor_reduce(
                                out=dst,
                                in0=ps[:],
                                in1=inf_t[:],
                                scale=1.0,
                                scalar=BIG,
                                op0=mn,
                                op1=mn,
                                accum_out=acc_s[:],
                            )
                        else:
                            nc.scalar.copy(dst, ps[:])

                    src = s_row
                    for d in range(1, fold + 1):
                        H = N >> d
                        rnew = rtmp_pool.tile([P, H], fp16, tag=f"fold{d}")
                        nc.vector.tensor_tensor(
                            rnew[:], src[:, :H], src[:, H:2 * H], op=mn
                        )
                        src = rnew
                    nc.sync.dma_start(r1_d[t], src[:])
                    if t in dmacol_tiles:
                        nc.gpsimd.dma_start(colacc_b[:], s_row[:], accum_op=mn)
                    else:
                        nc.vector.tensor_tensor(
                            colacc_a[:], colacc_a[:], s_row[:], op=mn
                        )

            nc.sync.dma_start(cola_d[:], colacc_a[:])
            if colacc_b is not None:
                nc.sync.dma_start(colb_d[:], colacc_b[:])

    nc.compile()
    return nc


def _get_nc():
    if "nc" not in _CACHE:
        _CACHE["nc"] = _build_module()
    return _CACHE["nc"]


def _split_bf16(x):
    hi = x.astype(_bf16)
    lo = (x - hi.astype(np.float32)).astype(_bf16)
    return hi, lo


def _build_operands(a, bpts):
    """a, bpts: [N, 3] fp32 -> (lhs [13,N] bf16, rhs [13,N] bf16)."""
    ahi, alo = _split_bf16(a)
    a2 = np.sum(a.astype(np.float64) ** 2, axis=1).astype(np.float32)
    a2hi, a2lo = _split_bf16(a2)
    bhi, blo = _split_bf16(bpts)
    b2 = np.sum(bpts.astype(np.float64) ** 2, axis=1).astype(np.float32)
    b2hi, b2lo = _split_bf16(b2)

    m2ahi = (-2.0 * ahi.astype(np.float32)).astype(_bf16)
    m2alo = (-2.0 * alo.astype(np.float32)).astype(_bf16)

    L = np.zeros((K, N), dtype=_bf16)
    R = np.zeros((K, N), dtype=_bf16)
    L[0:3] = m2ahi.T
    L[3:6] = m2ahi.T
    L[6:9] = m2alo.T
    L[9] = a2hi
    L[10] = a2lo
    L[11] = 1
    L[12] = 1
    R[0:3] = bhi.T
    R[3:6] = blo.T
    R[6:9] = bhi.T
    R[9] = 1
    R[10] = 1
    R[11] = b2hi
    R[12] = b2lo
    return L, R


def _make_in_maps(predict_pc, gt_pc):
    in_maps = []
    for b in range(B):
        a = np.ascontiguousarray(np.asarray(predict_pc[b], dtype=np.float32))
        g = np.ascontiguousarray(np.asarray(gt_pc[b], dtype=np.float32))
        L, R = _build_operands(a, g)
        in_maps.append({"lhs": L, "rhs": R})
    return in_maps


def _postprocess(results):
    total = 0.0
    for b in range(B):
        r1 = results[b]["r1"]                                    # [NT, P, N>>FOLD]
        ca = results[b]["colacc_a"].astype(np.float32)           # [P, N]
        if "colacc_b" in results[b]:
            ca = np.minimum(ca, results[b]["colacc_b"].astype(np.float32))
        rm = r1.astype(np.float32).min(axis=2).astype(np.float64)
        total += np.maximum(rm, 0.0).sum()
        colmin = ca.min(axis=0).astype(np.float64)
        total += np.maximum(colmin, 0.0).sum()
    return np.array(total / (2.0 * B * N), dtype=np.float32)


def kernel(predict_pc, gt_pc):
    from concourse import bass_utils

    nc = _get_nc()
    in_maps = _make_in_maps(predict_pc, gt_pc)
    res = bass_utils.run_bass_kernel_spmd(nc, in_maps, core_ids=list(range(B)))
    return _postprocess(res.results)


# revision 41
# speedup vs baseline: 1.1836x; 1.1836x over previous
"""Chamfer loss kernel for Trainium2 (8 NeuronCores, data-parallel over batch).

Strategy
--------
loss = (mean_i min_j d2[i,j] + mean_j min_i d2[i,j]) / 2 with
d2[i,j] = |a_i|^2 + |b_j|^2 - 2 a_i.b_j computed per batch (one batch per core).

Device kernel (per core, one batch, N=8192 points, D=3):
  * Host pre-builds augmented K=13 bf16 operands so a single TensorE matmul
    emits exact-enough fp32 d2 tiles straight into PSUM:
      rows 0-8: hi/lo split of -2*a paired with hi/lo of b (bf16 hi/lo
                double-float trick, ~2^-18 relative precision),
      rows 9-12: |a|^2 hi/lo against ones, ones against |b|^2 hi/lo.
  * The PSUM fp32 -> SBUF fp16 cast-copies (1024-wide chunks, 4-deep PSUM
    ring) are emitted as nc.any.tensor_copy so the Tile scheduler splits
    them between ScalarE and VectorE by modeled busy-ness; the two 1x-rate
    copy engines and the DMA engines all run ~95% occupied.
  * Row direction: the fp16 d2 rows ship to DRAM and the host takes the row
    min (the DMA engines have the spare bandwidth; VectorE does not).
  * Column direction: VectorE accumulates a running elementwise-min tile at
    the 2x fp16 perf mode.
  * Host (float64): row mins, cross-partition column min, clamp at 0, sum,
    normalize.
"""

import numpy as np
import ml_dtypes

B, N, D = 8, 8192, 3
K = 13          # augmented contraction rows
P = 128         # partitions / i-tile height
JC = 1024       # copy chunk (2 PSUM banks)
MM = 512        # matmul moving free dim (1 PSUM bank)
NT = N // P     # 64 i-tiles
NC_ = N // JC   # 4 chunks per i-tile row
NCHUNKS = NT * NC_  # 256 total copy chunks
N_TTR = 0       # chunks copied by VectorE (TTR) instead of ScalarE
N_DMACOL = 0    # accum-min DMA: rejected by walrus birverifier, keep 0
FOLD = 0        # row-min fold depth on device; r1 ships N >> FOLD wide
BIG = 60000.0   # fp16-safe +inf surrogate (d2 <= ~300 here)

_bf16 = ml_dtypes.bfloat16

_CACHE = {}


def _spread(n, total):
    # spread special-path items evenly
    if n <= 0:
        return set()
    step = total / n
    return {int(step * i + step / 2) for i in range(n)}


def _build_module(repeats=1, n_ttr=N_TTR, n_dmacol=N_DMACOL, jc=JC, fold=FOLD):
    import concourse.bass as bass
    import concourse.tile as tile
    from concourse import bacc, mybir

    nchunk_row = N // jc
    nchunks = NT * nchunk_row
    psum_bufs = 4096 // jc

    nc = bacc.Bacc(
        "TRN2",
        target_bir_lowering=False,
        debug=False,
        enable_asserts=False,
    )

    lhs_d = nc.dram_tensor("lhs", [K, N], mybir.dt.bfloat16, kind="ExternalInput")
    rhs_d = nc.dram_tensor("rhs", [K, N], mybir.dt.bfloat16, kind="ExternalInput")
    cola_d = nc.dram_tensor("colacc_a", [P, N], mybir.dt.float16, kind="ExternalOutput")
    colb_d = (
        nc.dram_tensor("colacc_b", [P, N], mybir.dt.float16, kind="ExternalOutput")
        if n_dmacol > 0
        else None
    )
    r1_d = nc.dram_tensor(
        "r1", [NT, P, N >> fold], mybir.dt.float16, kind="ExternalOutput"
    )

    fp16 = mybir.dt.float16
    f32 = mybir.dt.float32
    mn = mybir.AluOpType.min
    ttr_chunks = _spread(n_ttr, nchunks)
    dmacol_tiles = _spread(n_dmacol, NT)

    with tile.TileContext(nc) as tc:
        with (
            tc.tile_pool(name="const", bufs=1) as const,
            tc.tile_pool(name="srow", bufs=4) as srow_pool,
            tc.tile_pool(name="rtmp", bufs=3) as rtmp_pool,
            tc.tile_pool(name="accs", bufs=2) as accs_pool,
            tc.tile_pool(name="psum", bufs=psum_bufs, space="PSUM") as psum_pool,
        ):
            lhs_sb = const.tile([K, N], mybir.dt.bfloat16)
            rhs_sb = const.tile([K, N], mybir.dt.bfloat16)
            nc.sync.dma_start(lhs_sb[:], lhs_d[:])
            nc.sync.dma_start(rhs_sb[:], rhs_d[:])

            colacc_a = const.tile([P, N], fp16)
            colacc_b = const.tile([P, N], fp16) if n_dmacol > 0 else None
            inf_t = const.tile([P, jc], fp16)
            nc.gpsimd.memset(inf_t[:], BIG)

            # touch ScalarE immediately so its activation-table load and
            # first-op drain hide under the input DMAs
            warm = const.tile([1, 32], fp16)
            nc.vector.memset(warm[:], 0.0)
            nc.scalar.copy(warm[:], warm[:])

            for _rep in range(repeats):
                nc.gpsimd.memset(colacc_a[:], BIG)
                if colacc_b is not None:
                    nc.gpsimd.memset(colacc_b[:], BIG)
                for t in range(NT):
                    lhsT = lhs_sb[:, t * P:(t + 1) * P]
                    s_row = srow_pool.tile([P, N], fp16)
                    for c in range(nchunk_row):
                        ps = psum_pool.tile([P, jc], f32)
                        for q in range(jc // MM):
                            j0 = c * jc + q * MM
                            nc.tensor.matmul(
                                ps[:, q * MM:(q + 1) * MM],
                                lhsT,
                                rhs_sb[:, j0:j0 + MM],
                                start=True,
                                stop=True,
                            )
                        dst = s_row[:, c * jc:(c + 1) * jc]
                        if t * nchunk_row + c in ttr_chunks:
                            # fp32 PSUM -> fp16 SBUF copy on VectorE (engine
                            # balance); the row-min accum is a throwaway.
                            acc_s = accs_pool.tile([P, 1], f32)
                            nc.vector.tensor_tensor_reduce(
                                out=dst,
                                in0=ps[:],
                                in1=inf_t[:],
                                scale=1.0,
                                scalar=BIG,
                                op0=mn,
                                op1=mn,
                                accum_out=acc_s[:],
                            )
                        else:
                            # multi-engine: Tile routes each copy to ScalarE
                            # or VectorE by modeled busy-ness
                            nc.any.tensor_copy(dst, ps[:])

                    src = s_row
                    for d in range(1, fold + 1):
                        H = N >> d
                        rnew = rtmp_pool.tile([P, H], fp16, tag=f"fold{d}")
                        nc.vector.tensor_tensor(
                            rnew[:], src[:, :H], src[:, H:2 * H], op=mn
                        )
                        src = rnew
                    nc.sync.dma_start(r1_d[t], src[:])
                    if t in dmacol_tiles:
                        nc.gpsimd.dma_start(colacc_b[:], s_row[:], accum_op=mn)
                    else:
                        nc.vector.tensor_tensor(
                            colacc_a[:], colacc_a[:], s_row[:], op=mn
                        )

            nc.sync.dma_start(cola_d[:], colacc_a[:])
            if colacc_b is not None:
                nc.sync.dma_start(colb_d[:], colacc_b[:])

    nc.compile()
    return nc


def _get_nc():
    if "nc" not in _CACHE:
        _CACHE["nc"] = _build_module()
    return _CACHE["nc"]


def _split_bf16(x):
    hi = x.astype(_bf16)
    lo = (x - hi.astype(np.float32)).astype(_bf16)
    return hi, lo


def _build_operands(a, bpts):
    """a, bpts: [N, 3] fp32 -> (lhs [13,N] bf16, rhs [13,N] bf16)."""
    ahi, alo = _split_bf16(a)
    a2 = np.sum(a.astype(np.float64) ** 2, axis=1).astype(np.float32)
    a2hi, a2lo = _split_bf16(a2)
    bhi, blo = _split_bf16(bpts)
    b2 = np.sum(bpts.astype(np.float64) ** 2, axis=1).astype(np.float32)
    b2hi, b2lo = _split_bf16(b2)

    m2ahi = (-2.0 * ahi.astype(np.float32)).astype(_bf16)
    m2alo = (-2.0 * alo.astype(np.float32)).astype(_bf16)

    L = np.zeros((K, N), dtype=_bf16)
    R = np.zeros((K, N), dtype=_bf16)
    L[0:3] = m2ahi.T
    L[3:6] = m2ahi.T
    L[6:9] = m2alo.T
    L[9] = a2hi
    L[10] = a2lo
    L[11] = 1
    L[12] = 1
    R[0:3] = bhi.T
    R[3:6] = blo.T
    R[6:9] = bhi.T
    R[9] = 1
    R[10] = 1
    R[11] = b2hi
    R[12] = b2lo
    return L, R


def _make_in_maps(predict_pc, gt_pc):
    in_maps = []
    for b in range(B):
        a = np.ascontiguousarray(np.asarray(predict_pc[b], dtype=np.float32))
        g = np.ascontiguousarray(np.asarray(gt_pc[b], dtype=np.float32))
        L, R = _build_operands(a, g)
        in_maps.append({"lhs": L, "rhs": R})
    return in_maps


def _postprocess(results):
    total = 0.0
    for b in range(B):
        r1 = results[b]["r1"]                                    # [NT, P, N>>FOLD]
        ca = results[b]["colacc_a"].astype(np.float32)           # [P, N]
        if "colacc_b" in results[b]:
            ca = np.minimum(ca, results[b]["colacc_b"].astype(np.float32))
        rm = r1.astype(np.float32).min(axis=2).astype(np.float64)
        total += np.maximum(rm, 0.0).sum()
        colmin = ca.min(axis=0).astype(np.float64)
        total += np.maximum(colmin, 0.0).sum()
    return np.array(total / (2.0 * B * N), dtype=np.float32)


def kernel(predict_pc, gt_pc):
    from concourse import bass_utils

    nc = _get_nc()
    in_maps = _make_in_maps(predict_pc, gt_pc)
    res = bass_utils.run_bass_kernel_spmd(nc, in_maps, core_ids=list(range(B)))
    return _postprocess(res.results)
